# revision 2
# baseline (speedup 1.0000x reference)
"""Trainium2 Bass kernel for the MoE-routing module.

Computation (B=32768, D=1024, H=512, F=100, E=16, K=2):
    h   = relu(x @ W_shared + b_shared)                  [B, H]
    a   = relu(einsum('bh,ehf', h, W1) + b1)             [B, E, F]
    o   = einsum('bef,efo', a, W2) + b2                  [B, E, 1]
    out = mean over the K routed experts of o[b, send_to[idx[b]]]

Strategy: pure data-parallel over batch on 8 cores (4096 tokens each).
The routing is folded into a host-computed mask M[e, b] = (1/K) * count of
e among send_to[idx[b]], so the device computes
    out[b] = sum_e o[b, e] * M[e, b]
with three matmul stages, keeping features on SBUF partitions throughout:
  M1: hT[h, t]  = relu(W_shared.T @ xT)        lhsT=W_shared tiles
  M2: aT[f', t] = relu(W1cat.T @ hT)           f' = e*F + f  (E*F = 1600)
  M3: c[e, t]   = W2bd.T @ aT                  W2bd block-diagonal [1600, 16]
  sel: out[t]   = ones.T @ (c * mask)          1-partition result row
"""

import numpy as np

import concourse.mybir as mybir
from concourse import bacc
from concourse.bass_utils import run_bass_kernel_spmd
from concourse.tile import TileContext

B, D, H, F, E, TOPK = 32768, 1024, 512, 100, 16, 2
N_CORES = 8
BL = B // N_CORES          # tokens per core
CHUNK = 512                # tokens per device-side tile loop
N_CHUNKS = BL // CHUNK
EF = E * F                 # 1600
KT3 = (EF + 127) // 128    # 13 contraction tiles for M3
EF_PAD = KT3 * 128         # 1664

# Compute dtype for the matmul stages: "float32", "float32r", or "bfloat16"
COMPUTE_DT = "float32"

_FP32 = mybir.dt.float32
_cache = {}


def _np_in_dtype():
    import ml_dtypes

    return ml_dtypes.bfloat16 if COMPUTE_DT == "bfloat16" else np.float32


def _build_nc():
    CDT = getattr(mybir.dt, COMPUTE_DT)
    nc = bacc.Bacc("TRN2", target_bir_lowering=False, num_devices=N_CORES)

    xT_d = nc.declare_dram_parameter("xT", [D, BL], CDT, isOutput=False)
    mask_d = nc.declare_dram_parameter("mask", [E, BL], _FP32, isOutput=False)
    wsh_d = nc.declare_dram_parameter("wsh", [D, H], CDT, isOutput=False)
    bsh_d = nc.declare_dram_parameter("bsh", [H], _FP32, isOutput=False)
    w1c_d = nc.declare_dram_parameter("w1c", [H, EF], CDT, isOutput=False)
    b1f_d = nc.declare_dram_parameter("b1f", [EF_PAD], _FP32, isOutput=False)
    w2bd_d = nc.declare_dram_parameter("w2bd", [EF_PAD, E], CDT, isOutput=False)
    b2_d = nc.declare_dram_parameter("b2", [E], _FP32, isOutput=False)
    out_d = nc.declare_dram_parameter("out", [BL], _FP32, isOutput=True)

    KD = D // 128   # 8 contraction tiles for M1
    MH = H // 128   # 4 output tiles for M1
    KH = H // 128   # 4 contraction tiles for M2
    relu = mybir.ActivationFunctionType.Relu

    with TileContext(nc) as tc:
        with (
            tc.tile_pool(name="weights", bufs=1) as wpool,
            tc.tile_pool(name="xin", bufs=2) as xpool,
            tc.tile_pool(name="mid", bufs=2) as midpool,
            tc.tile_pool(name="small", bufs=2) as spool,
            tc.tile_pool(name="ps_h", bufs=2, space="PSUM") as ps_h,
            tc.tile_pool(name="ps_a", bufs=2, space="PSUM") as ps_a,
            tc.tile_pool(name="ps_c", bufs=2, space="PSUM") as ps_c,
            tc.tile_pool(name="ps_o", bufs=2, space="PSUM") as ps_o,
        ):
            # ---- resident weights / biases ----
            wsh_sb = wpool.tile([128, KD, H], CDT)
            nc.sync.dma_start(wsh_sb[:], wsh_d.rearrange("(o p) h -> p o h", p=128))
            w1c_sb = wpool.tile([128, KH, EF], CDT)
            nc.sync.dma_start(w1c_sb[:], w1c_d.rearrange("(o p) f -> p o f", p=128))
            w2bd_sb = wpool.tile([128, KT3, E], CDT)
            nc.sync.dma_start(w2bd_sb[:], w2bd_d.rearrange("(o p) e -> p o e", p=128))
            bsh_sb = wpool.tile([128, MH], _FP32)
            nc.sync.dma_start(bsh_sb[:], bsh_d.rearrange("(o p) -> p o", p=128))
            b1f_sb = wpool.tile([128, KT3], _FP32)
            nc.sync.dma_start(b1f_sb[:], b1f_d.rearrange("(o p) -> p o", p=128))
            b2_sb = wpool.tile([E, 1], _FP32)
            nc.sync.dma_start(b2_sb[:], b2_d.rearrange("(e o) -> e o", o=1))
            ones_sb = wpool.tile([E, 1], CDT)
            nc.vector.memset(ones_sb[:], 1.0)

            xT_view = xT_d.rearrange("(o p) t -> p o t", p=128)

            for c in range(N_CHUNKS):
                t0 = c * CHUNK
                # ---- load x^T chunk [128, KD, CHUNK] ----
                xt = xpool.tile([128, KD, CHUNK], CDT, tag="xt")
                nc.sync.dma_start(xt[:], xT_view[:, :, t0 : t0 + CHUNK])
                mask_sb = spool.tile([E, CHUNK], _FP32, tag="mask")
                nc.sync.dma_start(mask_sb[:], mask_d[:, t0 : t0 + CHUNK])

                # ---- M1: hT = relu(W_shared.T @ xT + b) ----
                hT = midpool.tile([128, MH, CHUNK], CDT, tag="hT")
                for m in range(MH):
                    ph = ps_h.tile([128, CHUNK], _FP32, tag="ps_h")
                    for k in range(KD):
                        nc.tensor.matmul(
                            ph[:],
                            lhsT=wsh_sb[:, k, m * 128 : (m + 1) * 128],
                            rhs=xt[:, k, :],
                            start=(k == 0),
                            stop=(k == KD - 1),
                        )
                    nc.scalar.activation(
                        hT[:, m, :], ph[:], relu, bias=bsh_sb[:, m : m + 1]
                    )

                # ---- M2: aT = relu(W1cat.T @ hT + b1) ----
                aT = midpool.tile([128, KT3, CHUNK], CDT, tag="aT")
                for m in range(KT3):
                    f0 = m * 128
                    fw = min(128, EF - f0)
                    pa = ps_a.tile([128, CHUNK], _FP32, tag="ps_a")
                    for k in range(KH):
                        nc.tensor.matmul(
                            pa[:fw],
                            lhsT=w1c_sb[:, k, f0 : f0 + fw],
                            rhs=hT[:, k, :],
                            start=(k == 0),
                            stop=(k == KH - 1),
                        )
                    nc.scalar.activation(
                        aT[:fw, m, :], pa[:fw], relu, bias=b1f_sb[:fw, m : m + 1]
                    )
                    if fw < 128:
                        nc.vector.memset(aT[fw:, m, :], 0.0)

                # ---- M3: c = W2bd.T @ aT  (block-diag W2) ----
                pc = ps_c.tile([E, CHUNK], _FP32, tag="ps_c")
                for k in range(KT3):
                    nc.tensor.matmul(
                        pc[:],
                        lhsT=w2bd_sb[:, k, :],
                        rhs=aT[:, k, :],
                        start=(k == 0),
                        stop=(k == KT3 - 1),
                    )

                # ---- select: out = ones.T @ ((c + b2) * mask) ----
                msel = spool.tile([E, CHUNK], CDT, tag="msel")
                nc.vector.tensor_scalar_add(msel[:], pc[:], b2_sb[:])
                nc.vector.tensor_mul(msel[:], msel[:], mask_sb[:])
                po = ps_o.tile([1, CHUNK], _FP32, tag="ps_o")
                nc.tensor.matmul(po[:], lhsT=ones_sb[:], rhs=msel[:], start=True, stop=True)
                ot = spool.tile([1, CHUNK], _FP32, tag="ot")
                nc.vector.tensor_copy(ot[:], po[:])
                nc.sync.dma_start(out_d[t0 : t0 + CHUNK].rearrange("(o t) -> o t", o=1), ot[:])

    nc.compile()
    return nc


def get_nc():
    key = COMPUTE_DT
    if key not in _cache:
        _cache[key] = _build_nc()
    return _cache[key]


def prepare_in_maps(inputs):
    """Host-side sharding + weight prep. Returns in_maps for 8 cores."""
    np_dt = _np_in_dtype()
    x = np.ascontiguousarray(np.asarray(inputs["x"], dtype=np.float32))
    idx = np.asarray(inputs["idx"]).astype(np.int64).reshape(B)
    W_shared = np.asarray(inputs["W_shared"], dtype=np.float32)
    b_shared = np.asarray(inputs["b_shared"], dtype=np.float32).reshape(H)
    W1 = np.asarray(inputs["W1"], dtype=np.float32)
    b1 = np.asarray(inputs["b1"], dtype=np.float32)
    W2 = np.asarray(inputs["W2"], dtype=np.float32)
    b2 = np.asarray(inputs["b2"], dtype=np.float32).reshape(E)
    send_to = np.asarray(inputs["send_to"]).astype(np.int64)

    # routing mask: mask[e, b] = (1/TOPK) * |{k : send_to[idx[b], k] == e}|
    routes = send_to[idx]  # [B, K]
    mask = np.zeros((E, B), dtype=np.float32)
    for k in range(routes.shape[1]):
        np.add.at(mask, (routes[:, k], np.arange(B)), 1.0 / routes.shape[1])

    w1c = np.ascontiguousarray(W1.transpose(1, 0, 2).reshape(H, EF)).astype(np_dt)
    b1f = np.zeros(EF_PAD, dtype=np.float32)
    b1f[:EF] = b1.reshape(EF)
    w2bd = np.zeros((EF_PAD, E), dtype=np.float32)
    for e in range(E):
        w2bd[e * F : (e + 1) * F, e] = W2[e, :, 0]
    w2bd = w2bd.astype(np_dt)
    wsh = np.ascontiguousarray(W_shared).astype(np_dt)

    in_maps = []
    for c in range(N_CORES):
        sl = slice(c * BL, (c + 1) * BL)
        in_maps.append(
            {
                "xT": np.ascontiguousarray(x[sl].T).astype(np_dt),
                "mask": np.ascontiguousarray(mask[:, sl]),
                "wsh": wsh,
                "bsh": b_shared,
                "w1c": w1c,
                "b1f": b1f,
                "w2bd": w2bd,
                "b2": b2,
            }
        )
    return in_maps


def kernel(**inputs) -> np.ndarray:
    nc = get_nc()
    in_maps = prepare_in_maps(inputs)
    res = run_bass_kernel_spmd(nc, in_maps, list(range(N_CORES)))
    out = np.concatenate([res.results[c]["out"] for c in range(N_CORES)])
    return out.reshape(B, 1).astype(np.float32)


# revision 5
# speedup vs baseline: 2.9983x; 2.9983x over previous
"""Trainium2 Bass kernel for the MoE-routing module.

Computation (B=32768, D=1024, H=512, F=100, E=16, K=2):
    h   = relu(x @ W_shared + b_shared)                  [B, H]
    a   = relu(einsum('bh,ehf', h, W1) + b1)             [B, E, F]
    o   = einsum('bef,efo', a, W2) + b2                  [B, E, 1]
    out = mean over the K routed experts of o[b, send_to[idx[b]]]

Strategy: pure data-parallel over batch on 8 cores (4096 tokens each).
The routing is folded into a host-computed mask M[e, b] = (1/K) * count of
e among send_to[idx[b]], so the device computes
    out[b] = sum_e o[b, e] * M[e, b]
with three matmul stages, keeping features on SBUF partitions throughout:
  M1: hT[h, t]  = relu(W_shared.T @ xT)        lhsT=W_shared tiles
  M2: aT[f', t] = relu(W1cat.T @ hT)           f' = e*F + f  (E*F = 1600)
  M3: c[e, t]   = W2bd.T @ aT                  W2bd block-diagonal [1600, 16]
  sel: out[t]   = ones.T @ (c * mask)          1-partition result row
"""

import numpy as np

import concourse.mybir as mybir
from concourse import bacc
from concourse.bass_utils import run_bass_kernel_spmd
from concourse.tile import TileContext

B, D, H, F, E, TOPK = 32768, 1024, 512, 100, 16, 2
N_CORES = 8
BL = B // N_CORES          # tokens per core
CHUNK = 512                # tokens per device-side tile loop
N_CHUNKS = BL // CHUNK
EF = E * F                 # 1600
KT3 = (EF + 127) // 128    # 13 contraction tiles for M3
EF_PAD = KT3 * 128         # 1664

# Compute dtype for the matmul stages: "float32", "float32r", or "bfloat16"
import os
COMPUTE_DT = os.environ.get("KERNEL_DT", "float32")

_FP32 = mybir.dt.float32
_cache = {}


def _np_in_dtype():
    import ml_dtypes

    return ml_dtypes.bfloat16 if COMPUTE_DT == "bfloat16" else np.float32


def _build_nc():
    # CDT: dtype of matmul-feeding tensors (x, weights, hT, aT).
    # SDT: dtype of the tiny select stage (mask-mult + ones-matmul).
    CDT = getattr(mybir.dt, COMPUTE_DT)
    SDT = mybir.dt.bfloat16 if COMPUTE_DT == "bfloat16" else mybir.dt.float32

    def mm(ap):
        return ap
    nc = bacc.Bacc("TRN2", target_bir_lowering=False, num_devices=N_CORES)

    xT_d = nc.declare_dram_parameter("xT", [D, BL], CDT, isOutput=False)
    mask_d = nc.declare_dram_parameter("mask", [E, BL], _FP32, isOutput=False)
    wsh_d = nc.declare_dram_parameter("wsh", [D, H], CDT, isOutput=False)
    bsh_d = nc.declare_dram_parameter("bsh", [H], _FP32, isOutput=False)
    w1c_d = nc.declare_dram_parameter("w1c", [H, EF], CDT, isOutput=False)
    b1f_d = nc.declare_dram_parameter("b1f", [EF_PAD], _FP32, isOutput=False)
    w2bd_d = nc.declare_dram_parameter("w2bd", [EF_PAD, E], CDT, isOutput=False)
    b2_d = nc.declare_dram_parameter("b2", [E], _FP32, isOutput=False)
    out_d = nc.declare_dram_parameter("out", [BL], _FP32, isOutput=True)

    KD = D // 128   # 8 contraction tiles for M1
    MH = H // 128   # 4 output tiles for M1
    KH = H // 128   # 4 contraction tiles for M2
    relu = mybir.ActivationFunctionType.Relu

    with TileContext(nc) as tc:
        with (
            tc.tile_pool(name="weights", bufs=1) as wpool,
            tc.tile_pool(name="xin", bufs=2) as xpool,
            tc.tile_pool(name="mid", bufs=2) as midpool,
            tc.tile_pool(name="small", bufs=2) as spool,
            tc.tile_pool(name="ps_h", bufs=2, space="PSUM") as ps_h,
            tc.tile_pool(name="ps_a", bufs=2, space="PSUM") as ps_a,
            tc.tile_pool(name="ps_c", bufs=2, space="PSUM") as ps_c,
            tc.tile_pool(name="ps_o", bufs=2, space="PSUM") as ps_o,
        ):
            # ---- resident weights / biases ----
            wsh_sb = wpool.tile([128, KD, H], CDT)
            nc.sync.dma_start(wsh_sb[:], wsh_d.rearrange("(o p) h -> p o h", p=128))
            w1c_sb = wpool.tile([128, KH, EF], CDT)
            nc.sync.dma_start(w1c_sb[:], w1c_d.rearrange("(o p) f -> p o f", p=128))
            w2bd_sb = wpool.tile([128, KT3, E], CDT)
            nc.sync.dma_start(w2bd_sb[:], w2bd_d.rearrange("(o p) e -> p o e", p=128))
            bsh_sb = wpool.tile([128, MH], _FP32)
            nc.sync.dma_start(bsh_sb[:], bsh_d.rearrange("(o p) -> p o", p=128))
            b1f_sb = wpool.tile([128, KT3], _FP32)
            nc.sync.dma_start(b1f_sb[:], b1f_d.rearrange("(o p) -> p o", p=128))
            b2_sb = wpool.tile([E, 1], _FP32)
            nc.sync.dma_start(b2_sb[:], b2_d.rearrange("(e o) -> e o", o=1))
            ones_sb = wpool.tile([E, 1], SDT)
            nc.vector.memset(ones_sb[:], 1.0)

            xT_view = xT_d.rearrange("(o p) t -> p o t", p=128)

            for c in range(N_CHUNKS):
                t0 = c * CHUNK
                # ---- load x^T chunk [128, KD, CHUNK] ----
                xt = xpool.tile([128, KD, CHUNK], CDT, tag="xt")
                nc.sync.dma_start(xt[:], xT_view[:, :, t0 : t0 + CHUNK])
                mask_sb = spool.tile([E, CHUNK], _FP32, tag="mask")
                nc.sync.dma_start(mask_sb[:], mask_d[:, t0 : t0 + CHUNK])

                # ---- M1: hT = relu(W_shared.T @ xT + b) ----
                hT = midpool.tile([128, MH, CHUNK], CDT, tag="hT")
                for m in range(MH):
                    ph = ps_h.tile([128, CHUNK], _FP32, tag="ps_h")
                    for k in range(KD):
                        nc.tensor.matmul(
                            ph[:],
                            lhsT=mm(wsh_sb[:, k, m * 128 : (m + 1) * 128]),
                            rhs=mm(xt[:, k, :]),
                            start=(k == 0),
                            stop=(k == KD - 1),
                        )
                    nc.scalar.activation(
                        hT[:, m, :], ph[:], relu, bias=bsh_sb[:, m : m + 1]
                    )

                # ---- M2: aT = relu(W1cat.T @ hT + b1) ----
                aT = midpool.tile([128, KT3, CHUNK], CDT, tag="aT")
                for m in range(KT3):
                    f0 = m * 128
                    fw = min(128, EF - f0)
                    pa = ps_a.tile([128, CHUNK], _FP32, tag="ps_a")
                    for k in range(KH):
                        nc.tensor.matmul(
                            pa[:fw],
                            lhsT=mm(w1c_sb[:, k, f0 : f0 + fw]),
                            rhs=mm(hT[:, k, :]),
                            start=(k == 0),
                            stop=(k == KH - 1),
                        )
                    nc.scalar.activation(
                        aT[:fw, m, :], pa[:fw], relu, bias=b1f_sb[:fw, m : m + 1]
                    )
                    if fw < 128:
                        nc.vector.memset(aT[fw:, m, :].bitcast(mybir.dt.float32), 0.0)

                # ---- M3: c = W2bd.T @ aT  (block-diag W2) ----
                pc = ps_c.tile([E, CHUNK], _FP32, tag="ps_c")
                for k in range(KT3):
                    nc.tensor.matmul(
                        pc[:],
                        lhsT=mm(w2bd_sb[:, k, :]),
                        rhs=mm(aT[:, k, :]),
                        start=(k == 0),
                        stop=(k == KT3 - 1),
                    )

                # ---- select: out = ones.T @ ((c + b2) * mask) ----
                msel = spool.tile([E, CHUNK], SDT, tag="msel")
                nc.vector.tensor_scalar_add(msel[:], pc[:], b2_sb[:])
                nc.vector.tensor_mul(msel[:], msel[:], mask_sb[:])
                po = ps_o.tile([1, CHUNK], _FP32, tag="ps_o")
                nc.tensor.matmul(po[:], lhsT=mm(ones_sb[:]), rhs=mm(msel[:]), start=True, stop=True)
                ot = spool.tile([1, CHUNK], _FP32, tag="ot")
                nc.vector.tensor_copy(ot[:], po[:])
                nc.sync.dma_start(out_d[t0 : t0 + CHUNK].rearrange("(o t) -> o t", o=1), ot[:])

    nc.compile()
    return nc


def get_nc():
    key = COMPUTE_DT
    if key not in _cache:
        _cache[key] = _build_nc()
    return _cache[key]


def prepare_in_maps(inputs):
    """Host-side sharding + weight prep. Returns in_maps for 8 cores."""
    np_dt = _np_in_dtype()
    x = np.ascontiguousarray(np.asarray(inputs["x"], dtype=np.float32))
    idx = np.asarray(inputs["idx"]).astype(np.int64).reshape(B)
    W_shared = np.asarray(inputs["W_shared"], dtype=np.float32)
    b_shared = np.asarray(inputs["b_shared"], dtype=np.float32).reshape(H)
    W1 = np.asarray(inputs["W1"], dtype=np.float32)
    b1 = np.asarray(inputs["b1"], dtype=np.float32)
    W2 = np.asarray(inputs["W2"], dtype=np.float32)
    b2 = np.asarray(inputs["b2"], dtype=np.float32).reshape(E)
    send_to = np.asarray(inputs["send_to"]).astype(np.int64)

    # routing mask: mask[e, b] = (1/TOPK) * |{k : send_to[idx[b], k] == e}|
    routes = send_to[idx]  # [B, K]
    mask = np.zeros((E, B), dtype=np.float32)
    for k in range(routes.shape[1]):
        np.add.at(mask, (routes[:, k], np.arange(B)), 1.0 / routes.shape[1])

    w1c = np.ascontiguousarray(W1.transpose(1, 0, 2).reshape(H, EF)).astype(np_dt)
    b1f = np.zeros(EF_PAD, dtype=np.float32)
    b1f[:EF] = b1.reshape(EF)
    w2bd = np.zeros((EF_PAD, E), dtype=np.float32)
    for e in range(E):
        w2bd[e * F : (e + 1) * F, e] = W2[e, :, 0]
    w2bd = w2bd.astype(np_dt)
    wsh = np.ascontiguousarray(W_shared).astype(np_dt)

    in_maps = []
    for c in range(N_CORES):
        sl = slice(c * BL, (c + 1) * BL)
        in_maps.append(
            {
                "xT": np.ascontiguousarray(x[sl].T).astype(np_dt),
                "mask": np.ascontiguousarray(mask[:, sl]),
                "wsh": wsh,
                "bsh": b_shared,
                "w1c": w1c,
                "b1f": b1f,
                "w2bd": w2bd,
                "b2": b2,
            }
        )
    return in_maps


def kernel(**inputs) -> np.ndarray:
    nc = get_nc()
    in_maps = prepare_in_maps(inputs)
    res = run_bass_kernel_spmd(nc, in_maps, list(range(N_CORES)))
    out = np.concatenate([res.results[c]["out"] for c in range(N_CORES)])
    return out.reshape(B, 1).astype(np.float32)
